# revision 14
# baseline (speedup 1.0000x reference)
"""Trainium2 Bass kernel for SAGAN-style spatial self-attention.

Reference computation (per batch b):
    xf = x[b].reshape(C, N)                    # C=256, N=64*64=4096
    f  = w1 @ xf                               # [32, N]   (query^T)
    g  = w2 @ xf                               # [32, N]   (key)
    V  = (w3 @ xf)^T                           # [N, C]    (value)
    S  = f^T @ g                               # [N, N]
    O  = softmax(S, axis=-1) @ V               # [N, C]
    out[b] = O^T.reshape(C, H, W) + x[b]

Sharding: 8 cores = 4 batches x 2 query-halves. Each core holds its batch's
full xf (for keys/values) and computes attention for 2048 query positions.
No cross-core communication.

Per-core device algorithm (n = this core's 2048 query cols, m = all 4096 keys):
  - projections f [32,2048], g [32,4096] in fp16; V [4096,257] in bf16
    (column 256 of V is ones -> PV matmul emits softmax denominator for free)
  - S^T chunks: matmul(lhsT=g_mtile [32,128], rhs=f_chunk [32,512]) -> PSUM
  - P^T = exp(S^T) -> SBUF bf16 (no max subtraction: |S| <~ 45, exp fits fp32)
  - O chunk: matmul(lhsT=P^T [128m,128n], rhs=V [128m,257]) accumulated over
    32 m-tiles -> [128n, 257]; r = 1/col256; O *= r (bf16)
  - DMA-transpose O to [C, n] layout, add residual xq, DMA out.

fp16 (not fp32/fp32r) operands everywhere on the PE: fp32-mode matmuls do not
register as PE-busy for the HAM clock gate and the PE gets stuck at 1.2GHz.
fp16 keeps full clock and has enough mantissa (2^-11) for the pre-exp scores.
"""

import sys

sys.path.insert(0, "/opt/trn_rl_repo")

from contextlib import ExitStack

import numpy as np

import concourse.bass as bass
import concourse.tile as tile
from concourse import bacc, mybir
from concourse.bass import ts, ds
from concourse.bass_utils import run_bass_kernel_spmd

F32 = mybir.dt.float32
F16 = mybir.dt.float16
BF16 = mybir.dt.bfloat16

B, C, H, W = 4, 256, 64, 64
N = H * W          # 4096 keys per batch
NQ = N // 2        # 2048 queries per core
CK = 32            # query/key head dim
MT = N // 128      # 32 m-tiles
NCHUNK = NQ // 512  # 4 n-chunks of 512 query cols
EXP = mybir.ActivationFunctionType.Exp


def build_nc():
    nc = bacc.Bacc("TRN2", target_bir_lowering=False, debug=False, num_devices=8)
    xkv_d = nc.dram_tensor("xkv", [C, N], F16, kind="ExternalInput")
    xq_d = nc.dram_tensor("xq", [C, NQ], F32, kind="ExternalInput")
    xqh_d = nc.dram_tensor("xqh", [C, NQ], F16, kind="ExternalInput")
    w1t_d = nc.dram_tensor("w1t", [C, CK], F16, kind="ExternalInput")
    w2t_d = nc.dram_tensor("w2t", [C, CK], F16, kind="ExternalInput")
    w3t_d = nc.dram_tensor("w3t", [C, C], F16, kind="ExternalInput")
    out_d = nc.dram_tensor("out", [C, NQ], F32, kind="ExternalOutput")

    with tile.TileContext(nc) as tc, ExitStack() as ctx:
        _body(ctx, tc, xkv_d.ap(), xq_d.ap(), xqh_d.ap(), w1t_d.ap(), w2t_d.ap(),
              w3t_d.ap(), out_d.ap())
    nc.compile()
    return nc


def _body(ctx, tc, xkv_d, xq_d, xqh_d, w1t_d, w2t_d, w3t_d, out_d):
    nc = tc.nc
    singles = ctx.enter_context(tc.tile_pool(name="singles", bufs=1))

    xq = singles.tile([128, 2, NQ], F32, tag="xq", name="xq")
    xkv_h = singles.tile([128, 2, N], F16, tag="xkv_h", name="xkv_h")
    xq_h = singles.tile([128, 2, NQ], F16, tag="xq_h", name="xq_h")
    w1t = singles.tile([128, 2, CK], F16, tag="w1t", name="w1t")
    w2t = singles.tile([128, 2, CK], F16, tag="w2t", name="w2t")
    w3t = singles.tile([128, 2, C], F16, tag="w3t", name="w3t")
    g_sb = singles.tile([CK, N], F16, tag="g_sb", name="g_sb")
    f_sb = singles.tile([CK, NQ], F16, tag="f_sb", name="f_sb")
    V = singles.tile([128, MT, 260], BF16, tag="V", name="V")

    nc.vector.memset(V[:, :, 256:257], 1.0)

    # PSUM: one shared pool ("st" tag, 2-bank slots, bufs=3) hosts the S^T
    # tiles and all projection outputs; one 1-bank pool (bufs=2) for the PV
    # accumulators. 6 + 2 = 8 banks.
    stp = ctx.enter_context(tc.tile_pool(name="st_ps", bufs=3, space="PSUM"))
    op = ctx.enter_context(tc.tile_pool(name="o_ps", bufs=2, space="PSUM"))
    ptp = ctx.enter_context(tc.tile_pool(name="pt", bufs=2))
    osbp = ctx.enter_context(tc.tile_pool(name="osb", bufs=2))
    otp = ctx.enter_context(tc.tile_pool(name="ot", bufs=4))
    rp = ctx.enter_context(tc.tile_pool(name="r", bufs=2))
    stgp = ctx.enter_context(tc.tile_pool(name="stage", bufs=2))

    Pt = [None, None]
    stage = [None, None]
    posts = []

    def emit_post(item):
        cc, j, o_ps, stg = item
        J = cc * 4 + j
        r = rp.tile([128, 1], F32, tag="r", name="r")
        nc.vector.reciprocal(r[:], o_ps[:, 256:257])
        o_sb = osbp.tile([128, 256], BF16, tag="osb", name="osb")
        nc.vector.tensor_scalar_mul(o_sb[:], o_ps[:, 0:256], r[:])
        for h in range(2):
            ot = otp.tile([128, 128], BF16, tag="ot", name="ot")
            nc.sync.dma_start_transpose(ot[:], o_sb[:, ts(h, 128)])
            nc.vector.tensor_add(stg[:, h, ts(j, 128)], ot[:],
                                 xq[:, h, ds(J * 128, 128)])
        for k in range(2):
            nc.gpsimd.dma_start(out_d[ts(k, 128), ds(J * 128, 128)],
                                stg[:, k, ts(j, 128)])

    def st_group(c, gidx):
        st = stp.tile([128, 2, 512], F32, tag="st", name="st")
        for t in range(2):
            mt = 2 * gidx + t
            nc.tensor.matmul(st[:, t, :], g_sb[:, ts(mt, 128)],
                             f_sb[:, ts(c, 512)], start=True, stop=True)
        nc.scalar.activation(Pt[c % 2][:, 2 * gidx:2 * gidx + 2, :], st[:], EXP)

    # ---- input DMAs (fp16 operands are cast host-side) ----
    for k in range(2):
        nc.sync.dma_start(w1t[:, k, :], w1t_d[ts(k, 128), :])
        nc.sync.dma_start(w2t[:, k, :], w2t_d[ts(k, 128), :])
        nc.sync.dma_start(w3t[:, k, :], w3t_d[ts(k, 128), :])
    for k in range(2):
        nc.sync.dma_start(xq_h[:, k, :], xqh_d[ts(k, 128), :])
    for half in range(2):
        for k in range(2):
            nc.sync.dma_start(xkv_h[:, k, ts(half, 2048)],
                              xkv_d[ts(k, 128), ts(half, 2048)])
    for k in range(2):
        nc.sync.dma_start(xq[:, k, :], xq_d[ts(k, 128), :])

    # ---- projections, interleaved with chunk 0 of the scores (S^T lags the
    # g-projection by one chunk so the PE never waits on the DVE g-copy) ----
    Pt[0] = ptp.tile([128, MT, 512], BF16, tag="pt", name="pt")
    for ch in range(NQ // 512):
        fp = stp.tile([CK, 512], F32, tag="st", name="fp")
        for k in range(2):
            nc.tensor.matmul(fp[:], w1t[:, k, :], xq_h[:, k, ts(ch, 512)],
                             start=(k == 0), stop=(k == 1))
        nc.vector.tensor_copy(f_sb[:, ts(ch, 512)], fp[:])
    for ch in range(N // 512):
        gp = stp.tile([CK, 512], F32, tag="st", name="gp")
        for k in range(2):
            nc.tensor.matmul(gp[:], w2t[:, k, :], xkv_h[:, k, ts(ch, 512)],
                             start=(k == 0), stop=(k == 1))
        nc.vector.tensor_copy(g_sb[:, ts(ch, 512)], gp[:])
        if ch >= 1:
            st_group(0, 2 * (ch - 1))
            st_group(0, 2 * ch - 1)
    st_group(0, 14)
    st_group(0, 15)
    for mt in range(MT):
        vp = stp.tile([128, 256], F32, tag="st", name="vp")
        for k in range(2):
            nc.tensor.matmul(vp[:], xkv_h[:, k, ts(mt, 128)], w3t[:, k, :],
                             start=(k == 0), stop=(k == 1))
        nc.vector.tensor_copy(V[:, mt, 0:256], vp[:])

    # ---- attention chunks 1..NCHUNK, software-pipelined by one chunk ----
    for c in range(1, NCHUNK + 1):
        if c < NCHUNK:
            Pt[c % 2] = ptp.tile([128, MT, 512], BF16, tag="pt", name="pt")
        stage[(c - 1) % 2] = stgp.tile([128, 2, 512], F32, tag="stage", name="stage")
        o_cur = None
        for gidx in range(16):
            if c < NCHUNK:
                st_group(c, gidx)
            j, seg = gidx // 4, gidx % 4
            if seg == 0:
                o_cur = op.tile([128, 257], F32, tag="o", name="o")
            for mm in range(8):
                mt = seg * 8 + mm
                nc.tensor.matmul(o_cur[:], Pt[(c - 1) % 2][:, mt, ts(j, 128)],
                                 V[:, mt, 0:257],
                                 start=(mt == 0), stop=(mt == MT - 1),
                                 skip_group_check=True)
            if seg == 3:
                posts.append((c - 1, j, o_cur, stage[(c - 1) % 2]))
            # delay each n-tile's post-processing by one PE group so the DVE
            # normalize never stalls the PE stream (flush at the end)
            while len(posts) > (1 if c <= NCHUNK - 1 or gidx < 15 else 0):
                emit_post(posts.pop(0))
    while posts:
        emit_post(posts.pop(0))


_NC_CACHE = None


def _get_nc():
    global _NC_CACHE
    if _NC_CACHE is None:
        _NC_CACHE = build_nc()
    return _NC_CACHE


def make_in_maps(x, w1, w2, w3):
    x = np.ascontiguousarray(x, dtype=np.float32).reshape(B, C, N)
    w1t = np.ascontiguousarray(w1.T, dtype=np.float32)
    w2t = np.ascontiguousarray(w2.T, dtype=np.float32)
    w3t = np.ascontiguousarray(w3.T, dtype=np.float32)
    in_maps = []
    xh = x.astype(np.float16)
    for core in range(8):
        b, half = core // 2, core % 2
        xq_core = np.ascontiguousarray(x[b][:, half * NQ:(half + 1) * NQ])
        in_maps.append({
            "xkv": xh[b],
            "xq": xq_core,
            "xqh": np.ascontiguousarray(xh[b][:, half * NQ:(half + 1) * NQ]),
            "w1t": w1t.astype(np.float16),
            "w2t": w2t.astype(np.float16),
            "w3t": w3t.astype(np.float16),
        })
    return in_maps


def assemble(results):
    out = np.empty((B, C, N), dtype=np.float32)
    for core in range(8):
        b, half = core // 2, core % 2
        out[b][:, half * NQ:(half + 1) * NQ] = results[core]["out"]
    return out.reshape(B, C, H, W)


def kernel(x, w1, w2, w3):
    nc = _get_nc()
    res = run_bass_kernel_spmd(nc, make_in_maps(x, w1, w2, w3),
                               core_ids=list(range(8)))
    return assemble(res.results)


# revision 17
# speedup vs baseline: 1.1262x; 1.1262x over previous
"""Trainium2 Bass kernel for SAGAN-style spatial self-attention.

Reference computation (per batch b):
    xf = x[b].reshape(C, N)                    # C=256, N=64*64=4096
    f  = w1 @ xf                               # [32, N]   (query^T)
    g  = w2 @ xf                               # [32, N]   (key)
    V  = (w3 @ xf)^T                           # [N, C]    (value)
    S  = f^T @ g                               # [N, N]
    O  = softmax(S, axis=-1) @ V               # [N, C]
    out[b] = O^T.reshape(C, H, W) + x[b]

Sharding: 8 cores = 4 batches x 2 query-halves. Each core holds its batch's
full xf (for keys/values) and computes attention for 2048 query positions.
No cross-core communication.

Per-core device algorithm (n = this core's 2048 query cols, m = all 4096 keys):
  - projections f [32,2048], g [32,4096] in fp16; V [4096,257] in bf16
    (column 256 of V is ones -> PV matmul emits softmax denominator for free)
  - S^T chunks: matmul(lhsT=g_mtile [32,128], rhs=f_chunk [32,512]) -> PSUM
  - P^T = exp(S^T) -> SBUF bf16 (no max subtraction: |S| <~ 45, exp fits fp32)
  - O chunk: matmul(lhsT=P^T [128m,128n], rhs=V [128m,257]) accumulated over
    32 m-tiles -> [128n, 257]; r = 1/col256; O *= r (bf16)
  - DMA-transpose O to [C, n] layout, add residual xq, DMA out.

fp16 (not fp32/fp32r) operands everywhere on the PE: fp32-mode matmuls do not
register as PE-busy for the HAM clock gate and the PE gets stuck at 1.2GHz.
fp16 keeps full clock and has enough mantissa (2^-11) for the pre-exp scores.
"""

import sys

sys.path.insert(0, "/opt/trn_rl_repo")

from contextlib import ExitStack

import numpy as np

import concourse.bass as bass
import concourse.tile as tile
from concourse import bacc, mybir
from concourse.bass import ts, ds
from concourse.bass_utils import run_bass_kernel_spmd

F32 = mybir.dt.float32
F16 = mybir.dt.float16
BF16 = mybir.dt.bfloat16

B, C, H, W = 4, 256, 64, 64
N = H * W          # 4096 keys per batch
NQ = N // 2        # 2048 queries per core
CK = 32            # query/key head dim
MT = N // 128      # 32 m-tiles
NCHUNK = NQ // 512  # 4 n-chunks of 512 query cols
EXP = mybir.ActivationFunctionType.Exp


def build_nc():
    nc = bacc.Bacc("TRN2", target_bir_lowering=False, debug=False, num_devices=8)
    xkv_d = nc.dram_tensor("xkv", [C, N], F16, kind="ExternalInput")
    xq_d = nc.dram_tensor("xq", [C, NQ], F32, kind="ExternalInput")
    xqh_d = nc.dram_tensor("xqh", [C, NQ], F16, kind="ExternalInput")
    w1t_d = nc.dram_tensor("w1t", [C, CK], F16, kind="ExternalInput")
    w2t_d = nc.dram_tensor("w2t", [C, CK], F16, kind="ExternalInput")
    w3t_d = nc.dram_tensor("w3t", [C, C], F16, kind="ExternalInput")
    out_d = nc.dram_tensor("out", [C, NQ], F32, kind="ExternalOutput")

    with tile.TileContext(nc) as tc, ExitStack() as ctx:
        _body(ctx, tc, xkv_d.ap(), xq_d.ap(), xqh_d.ap(), w1t_d.ap(), w2t_d.ap(),
              w3t_d.ap(), out_d.ap())
    nc.compile()
    return nc


def _body(ctx, tc, xkv_d, xq_d, xqh_d, w1t_d, w2t_d, w3t_d, out_d):
    nc = tc.nc
    singles = ctx.enter_context(tc.tile_pool(name="singles", bufs=1))

    xq = singles.tile([128, 2, NQ], F32, tag="xq", name="xq")
    xkv_h = singles.tile([128, 2, N], F16, tag="xkv_h", name="xkv_h")
    xq_h = singles.tile([128, 2, NQ], F16, tag="xq_h", name="xq_h")
    w1t = singles.tile([128, 2, CK], F16, tag="w1t", name="w1t")
    w2t = singles.tile([128, 2, CK], F16, tag="w2t", name="w2t")
    w3t = singles.tile([128, 2, C], F16, tag="w3t", name="w3t")
    g_sb = singles.tile([CK, N], F16, tag="g_sb", name="g_sb")
    f_sb = singles.tile([CK, NQ], F16, tag="f_sb", name="f_sb")
    V = singles.tile([128, MT, 260], BF16, tag="V", name="V")

    nc.vector.memset(V[:, :, 256:257], 1.0)

    # PSUM: one shared pool ("st" tag, 2-bank slots, bufs=3) hosts the S^T
    # tiles and all projection outputs; one 1-bank pool (bufs=2) for the PV
    # accumulators. 6 + 2 = 8 banks.
    stp = ctx.enter_context(tc.tile_pool(name="st_ps", bufs=3, space="PSUM"))
    op = ctx.enter_context(tc.tile_pool(name="o_ps", bufs=2, space="PSUM"))
    ptp = ctx.enter_context(tc.tile_pool(name="pt", bufs=2))
    osbp = ctx.enter_context(tc.tile_pool(name="osb", bufs=2))
    otp = ctx.enter_context(tc.tile_pool(name="ot", bufs=4))
    rp = ctx.enter_context(tc.tile_pool(name="r", bufs=2))
    stgp = ctx.enter_context(tc.tile_pool(name="stage", bufs=3))

    Pt = [None, None]
    stage = [None, None]
    posts = []

    def emit_post(item):
        cc, j, o_ps, stg = item
        J = cc * 4 + j
        r = rp.tile([128, 1], F32, tag="r", name="r")
        nc.vector.reciprocal(r[:], o_ps[:, 256:257])
        o_sb = osbp.tile([128, 256], BF16, tag="osb", name="osb")
        nc.vector.tensor_scalar_mul(o_sb[:], o_ps[:, 0:256], r[:])
        for h in range(2):
            ot = otp.tile([128, 128], BF16, tag="ot", name="ot")
            nc.sync.dma_start_transpose(ot[:], o_sb[:, ts(h, 128)])
            nc.vector.tensor_add(stg[:, h, ts(j, 128)], ot[:],
                                 xq[:, h, ds(J * 128, 128)])
        if j == 3:
            for k in range(2):
                nc.gpsimd.dma_start(out_d[ts(k, 128), ts(cc, 512)], stg[:, k, :])

    def st_group(c, gidx):
        st = stp.tile([128, 2, 512], F32, tag="st", name="st")
        for t in range(2):
            mt = 2 * gidx + t
            nc.tensor.matmul(st[:, t, :], g_sb[:, ts(mt, 128)],
                             f_sb[:, ts(c, 512)], start=True, stop=True)
        nc.scalar.activation(Pt[c % 2][:, 2 * gidx:2 * gidx + 2, :], st[:], EXP)

    # ---- input DMAs (fp16 operands are cast host-side) ----
    for k in range(2):
        nc.sync.dma_start(w1t[:, k, :], w1t_d[ts(k, 128), :])
        nc.sync.dma_start(w2t[:, k, :], w2t_d[ts(k, 128), :])
        nc.sync.dma_start(w3t[:, k, :], w3t_d[ts(k, 128), :])
    for ch in range(NQ // 512):
        for k in range(2):
            nc.sync.dma_start(xq_h[:, k, ts(ch, 512)], xqh_d[ts(k, 128), ts(ch, 512)])
    for ch in range(N // 512):
        for k in range(2):
            nc.sync.dma_start(xkv_h[:, k, ts(ch, 512)],
                              xkv_d[ts(k, 128), ts(ch, 512)])
    for k in range(2):
        nc.sync.dma_start(xq[:, k, :], xq_d[ts(k, 128), :])

    # ---- projections, interleaved with chunk 0 of the scores (S^T lags the
    # g-projection by one chunk so the PE never waits on the DVE g-copy) ----
    Pt[0] = ptp.tile([128, MT, 512], BF16, tag="pt", name="pt")
    for ch in range(NQ // 512):
        fp = stp.tile([CK, 512], F32, tag="st", name="fp")
        for k in range(2):
            nc.tensor.matmul(fp[:], w1t[:, k, :], xq_h[:, k, ts(ch, 512)],
                             start=(k == 0), stop=(k == 1))
        nc.vector.tensor_copy(f_sb[:, ts(ch, 512)], fp[:])
    for ch in range(N // 512):
        gp = stp.tile([CK, 512], F32, tag="st", name="gp")
        for k in range(2):
            nc.tensor.matmul(gp[:], w2t[:, k, :], xkv_h[:, k, ts(ch, 512)],
                             start=(k == 0), stop=(k == 1))
        nc.vector.tensor_copy(g_sb[:, ts(ch, 512)], gp[:])
        if ch >= 1:
            st_group(0, 2 * (ch - 1))
            st_group(0, 2 * ch - 1)
    st_group(0, 14)
    st_group(0, 15)
    for mt in range(MT):
        vp = stp.tile([128, 256], F32, tag="st", name="vp")
        for k in range(2):
            nc.tensor.matmul(vp[:], xkv_h[:, k, ts(mt, 128)], w3t[:, k, :],
                             start=(k == 0), stop=(k == 1))
        nc.vector.tensor_copy(V[:, mt, 0:256], vp[:])

    # ---- attention chunks 1..NCHUNK, software-pipelined by one chunk ----
    for c in range(1, NCHUNK + 1):
        if c < NCHUNK:
            Pt[c % 2] = ptp.tile([128, MT, 512], BF16, tag="pt", name="pt")
        stage[(c - 1) % 2] = stgp.tile([128, 2, 512], F32, tag="stage", name="stage")
        o_cur = None
        for gidx in range(16):
            if c < NCHUNK:
                st_group(c, gidx)
            j, seg = gidx // 4, gidx % 4
            if seg == 0:
                o_cur = op.tile([128, 257], F32, tag="o", name="o")
            for mm in range(8):
                mt = seg * 8 + mm
                nc.tensor.matmul(o_cur[:], Pt[(c - 1) % 2][:, mt, ts(j, 128)],
                                 V[:, mt, 0:257],
                                 start=(mt == 0), stop=(mt == MT - 1),
                                 skip_group_check=True)
            if seg == 3:
                posts.append((c - 1, j, o_cur, stage[(c - 1) % 2]))
            # delay each n-tile's post-processing by one PE group so the DVE
            # normalize never stalls the PE stream (flush at the end)
            while len(posts) > (1 if c <= NCHUNK - 1 or gidx < 15 else 0):
                emit_post(posts.pop(0))
    while posts:
        emit_post(posts.pop(0))


_NC_CACHE = None


def _get_nc():
    global _NC_CACHE
    if _NC_CACHE is None:
        _NC_CACHE = build_nc()
    return _NC_CACHE


def make_in_maps(x, w1, w2, w3):
    x = np.ascontiguousarray(x, dtype=np.float32).reshape(B, C, N)
    w1t = np.ascontiguousarray(w1.T, dtype=np.float32)
    w2t = np.ascontiguousarray(w2.T, dtype=np.float32)
    w3t = np.ascontiguousarray(w3.T, dtype=np.float32)
    in_maps = []
    xh = x.astype(np.float16)
    for core in range(8):
        b, half = core // 2, core % 2
        xq_core = np.ascontiguousarray(x[b][:, half * NQ:(half + 1) * NQ])
        in_maps.append({
            "xkv": xh[b],
            "xq": xq_core,
            "xqh": np.ascontiguousarray(xh[b][:, half * NQ:(half + 1) * NQ]),
            "w1t": w1t.astype(np.float16),
            "w2t": w2t.astype(np.float16),
            "w3t": w3t.astype(np.float16),
        })
    return in_maps


def assemble(results):
    out = np.empty((B, C, N), dtype=np.float32)
    for core in range(8):
        b, half = core // 2, core % 2
        out[b][:, half * NQ:(half + 1) * NQ] = results[core]["out"]
    return out.reshape(B, C, H, W)


def kernel(x, w1, w2, w3):
    nc = _get_nc()
    res = run_bass_kernel_spmd(nc, make_in_maps(x, w1, w2, w3),
                               core_ids=list(range(8)))
    return assemble(res.results)


# revision 18
# speedup vs baseline: 1.1274x; 1.0010x over previous
"""Trainium2 Bass kernel for SAGAN-style spatial self-attention.

Reference computation (per batch b):
    xf = x[b].reshape(C, N)                    # C=256, N=64*64=4096
    f  = w1 @ xf                               # [32, N]   (query^T)
    g  = w2 @ xf                               # [32, N]   (key)
    V  = (w3 @ xf)^T                           # [N, C]    (value)
    S  = f^T @ g                               # [N, N]
    O  = softmax(S, axis=-1) @ V               # [N, C]
    out[b] = O^T.reshape(C, H, W) + x[b]

Sharding: 8 cores = 4 batches x 2 query-halves. Each core holds its batch's
full xf (for keys/values) and computes attention for 2048 query positions.
No cross-core communication.

Per-core device algorithm (n = this core's 2048 query cols, m = all 4096 keys):
  - projections f [32,2048], g [32,4096] in fp16; V [4096,257] in bf16
    (column 256 of V is ones -> PV matmul emits softmax denominator for free)
  - S^T chunks: matmul(lhsT=g_mtile [32,128], rhs=f_chunk [32,512]) -> PSUM
  - P^T = exp(S^T) -> SBUF bf16 (no max subtraction: |S| <~ 45, exp fits fp32)
  - O chunk: matmul(lhsT=P^T [128m,128n], rhs=V [128m,257]) accumulated over
    32 m-tiles -> [128n, 257]; r = 1/col256; O *= r (bf16)
  - DMA-transpose O to [C, n] layout, add residual xq, DMA out.

fp16 (not fp32/fp32r) operands everywhere on the PE: fp32-mode matmuls do not
register as PE-busy for the HAM clock gate and the PE gets stuck at 1.2GHz.
fp16 keeps full clock and has enough mantissa (2^-11) for the pre-exp scores.
"""

import sys

sys.path.insert(0, "/opt/trn_rl_repo")

from contextlib import ExitStack

import numpy as np

import concourse.bass as bass
import concourse.tile as tile
from concourse import bacc, mybir
from concourse.bass import ts, ds
from concourse.bass_utils import run_bass_kernel_spmd

F32 = mybir.dt.float32
F16 = mybir.dt.float16
BF16 = mybir.dt.bfloat16

B, C, H, W = 4, 256, 64, 64
N = H * W          # 4096 keys per batch
NQ = N // 2        # 2048 queries per core
CK = 32            # query/key head dim
MT = N // 128      # 32 m-tiles
NCHUNK = NQ // 512  # 4 n-chunks of 512 query cols
EXP = mybir.ActivationFunctionType.Exp


def build_nc():
    nc = bacc.Bacc("TRN2", target_bir_lowering=False, debug=False, num_devices=8)
    xkv_d = nc.dram_tensor("xkv", [C, N], F16, kind="ExternalInput")
    xq_d = nc.dram_tensor("xq", [C, NQ], F32, kind="ExternalInput")
    xqh_d = nc.dram_tensor("xqh", [C, NQ], F16, kind="ExternalInput")
    w1t_d = nc.dram_tensor("w1t", [C, CK], F16, kind="ExternalInput")
    w2t_d = nc.dram_tensor("w2t", [C, CK], F16, kind="ExternalInput")
    w3t_d = nc.dram_tensor("w3t", [C, C], F16, kind="ExternalInput")
    out_d = nc.dram_tensor("out", [C, NQ], F32, kind="ExternalOutput")

    with tile.TileContext(nc) as tc, ExitStack() as ctx:
        _body(ctx, tc, xkv_d.ap(), xq_d.ap(), xqh_d.ap(), w1t_d.ap(), w2t_d.ap(),
              w3t_d.ap(), out_d.ap())
    nc.compile()
    return nc


def _body(ctx, tc, xkv_d, xq_d, xqh_d, w1t_d, w2t_d, w3t_d, out_d):
    nc = tc.nc
    singles = ctx.enter_context(tc.tile_pool(name="singles", bufs=1))

    xq = singles.tile([128, 2, NQ], F32, tag="xq", name="xq")
    xkv_h = singles.tile([128, 2, N], F16, tag="xkv_h", name="xkv_h")
    xq_h = singles.tile([128, 2, NQ], F16, tag="xq_h", name="xq_h")
    w1t = singles.tile([128, 2, CK], F16, tag="w1t", name="w1t")
    w2t = singles.tile([128, 2, CK], F16, tag="w2t", name="w2t")
    w3t = singles.tile([128, 2, C], F16, tag="w3t", name="w3t")
    g_sb = singles.tile([CK, N], F16, tag="g_sb", name="g_sb")
    f_sb = singles.tile([CK, NQ], F16, tag="f_sb", name="f_sb")
    V = singles.tile([128, MT, 260], BF16, tag="V", name="V")

    nc.vector.memset(V[:, :, 256:257], 1.0)

    # PSUM: one shared pool ("st" tag, 2-bank slots, bufs=3) hosts the S^T
    # tiles and all projection outputs; one 1-bank pool (bufs=2) for the PV
    # accumulators. 6 + 2 = 8 banks.
    stp = ctx.enter_context(tc.tile_pool(name="st_ps", bufs=3, space="PSUM"))
    op = ctx.enter_context(tc.tile_pool(name="o_ps", bufs=2, space="PSUM"))
    ptp = ctx.enter_context(tc.tile_pool(name="pt", bufs=2))
    osbp = ctx.enter_context(tc.tile_pool(name="osb", bufs=2))
    otp = ctx.enter_context(tc.tile_pool(name="ot", bufs=4))
    rp = ctx.enter_context(tc.tile_pool(name="r", bufs=2))
    stgp = ctx.enter_context(tc.tile_pool(name="stage", bufs=3))

    Pt = [None, None]
    stage = [None, None]
    posts = []

    def emit_post(item):
        cc, j, o_ps, stg = item
        J = cc * 4 + j
        r = rp.tile([128, 1], F32, tag="r", name="r")
        nc.vector.reciprocal(r[:], o_ps[:, 256:257])
        o_sb = osbp.tile([128, 256], BF16, tag="osb", name="osb")
        nc.vector.tensor_scalar_mul(o_sb[:], o_ps[:, 0:256], r[:])
        for h in range(2):
            ot = otp.tile([128, 128], BF16, tag="ot", name="ot")
            nc.sync.dma_start_transpose(ot[:], o_sb[:, ts(h, 128)])
            nc.vector.tensor_add(stg[:, h, ts(j, 128)], ot[:],
                                 xq[:, h, ds(J * 128, 128)])
        if j == 3:
            for k in range(2):
                nc.gpsimd.dma_start(out_d[ts(k, 128), ts(cc, 512)], stg[:, k, :])

    def st_group(c, gidx):
        st = stp.tile([128, 2, 512], F32, tag="st", name="st")
        for t in range(2):
            mt = 2 * gidx + t
            nc.tensor.matmul(st[:, t, :], g_sb[:, ts(mt, 128)],
                             f_sb[:, ts(c, 512)], start=True, stop=True)
        nc.scalar.activation(Pt[c % 2][:, 2 * gidx:2 * gidx + 2, :], st[:], EXP)

    # HAM warmup: the PE clock-gate opens only after ~3.4us of gapless
    # streaming; run a dummy dense bf16 burst while the input DMAs land so
    # the projection phase starts at 2.4GHz instead of 1.2GHz.
    warm = singles.tile([128, 512], BF16, tag="warm", name="warm")
    nc.vector.memset(warm[:], 0.0)
    wps = stp.tile([128, 2, 512], F32, tag="st", name="wps")
    for i in range(40):
        nc.tensor.matmul(wps[:, i % 2, :], warm[:, 0:128], warm[:],
                         start=True, stop=True)

    # ---- input DMAs (fp16 operands are cast host-side) ----
    for k in range(2):
        nc.sync.dma_start(w1t[:, k, :], w1t_d[ts(k, 128), :])
    for k in range(2):
        nc.sync.dma_start(xq_h[:, k, 0:512], xqh_d[ts(k, 128), 0:512])
    for k in range(2):
        nc.sync.dma_start(w2t[:, k, :], w2t_d[ts(k, 128), :])
        nc.sync.dma_start(w3t[:, k, :], w3t_d[ts(k, 128), :])
    for ch in range(1, NQ // 512):
        for k in range(2):
            nc.sync.dma_start(xq_h[:, k, ts(ch, 512)], xqh_d[ts(k, 128), ts(ch, 512)])
    for ch in range(N // 512):
        for k in range(2):
            nc.sync.dma_start(xkv_h[:, k, ts(ch, 512)],
                              xkv_d[ts(k, 128), ts(ch, 512)])
    for k in range(2):
        nc.sync.dma_start(xq[:, k, :], xq_d[ts(k, 128), :])

    # ---- projections, interleaved with chunk 0 of the scores (S^T lags the
    # g-projection by one chunk so the PE never waits on the DVE g-copy) ----
    Pt[0] = ptp.tile([128, MT, 512], BF16, tag="pt", name="pt")
    for ch in range(NQ // 512):
        fp = stp.tile([CK, 512], F32, tag="st", name="fp")
        for k in range(2):
            nc.tensor.matmul(fp[:], w1t[:, k, :], xq_h[:, k, ts(ch, 512)],
                             start=(k == 0), stop=(k == 1))
        nc.vector.tensor_copy(f_sb[:, ts(ch, 512)], fp[:])
    for ch in range(N // 512):
        gp = stp.tile([CK, 512], F32, tag="st", name="gp")
        for k in range(2):
            nc.tensor.matmul(gp[:], w2t[:, k, :], xkv_h[:, k, ts(ch, 512)],
                             start=(k == 0), stop=(k == 1))
        nc.vector.tensor_copy(g_sb[:, ts(ch, 512)], gp[:])
        if ch >= 1:
            st_group(0, 2 * (ch - 1))
            st_group(0, 2 * ch - 1)
    st_group(0, 14)
    st_group(0, 15)
    for mt in range(MT):
        vp = stp.tile([128, 256], F32, tag="st", name="vp")
        for k in range(2):
            nc.tensor.matmul(vp[:], xkv_h[:, k, ts(mt, 128)], w3t[:, k, :],
                             start=(k == 0), stop=(k == 1))
        nc.vector.tensor_copy(V[:, mt, 0:256], vp[:])

    # ---- attention chunks 1..NCHUNK, software-pipelined by one chunk ----
    for c in range(1, NCHUNK + 1):
        if c < NCHUNK:
            Pt[c % 2] = ptp.tile([128, MT, 512], BF16, tag="pt", name="pt")
        stage[(c - 1) % 2] = stgp.tile([128, 2, 512], F32, tag="stage", name="stage")
        o_cur = None
        for gidx in range(16):
            if c < NCHUNK:
                st_group(c, gidx)
            j, seg = gidx // 4, gidx % 4
            if seg == 0:
                o_cur = op.tile([128, 257], F32, tag="o", name="o")
            for mm in range(8):
                mt = seg * 8 + mm
                nc.tensor.matmul(o_cur[:], Pt[(c - 1) % 2][:, mt, ts(j, 128)],
                                 V[:, mt, 0:257],
                                 start=(mt == 0), stop=(mt == MT - 1),
                                 skip_group_check=True)
            if seg == 3:
                posts.append((c - 1, j, o_cur, stage[(c - 1) % 2]))
            # delay each n-tile's post-processing by one PE group so the DVE
            # normalize never stalls the PE stream (flush at the end)
            while len(posts) > (1 if c <= NCHUNK - 1 or gidx < 15 else 0):
                emit_post(posts.pop(0))
    while posts:
        emit_post(posts.pop(0))


_NC_CACHE = None


def _get_nc():
    global _NC_CACHE
    if _NC_CACHE is None:
        _NC_CACHE = build_nc()
    return _NC_CACHE


def make_in_maps(x, w1, w2, w3):
    x = np.ascontiguousarray(x, dtype=np.float32).reshape(B, C, N)
    w1t = np.ascontiguousarray(w1.T, dtype=np.float32)
    w2t = np.ascontiguousarray(w2.T, dtype=np.float32)
    w3t = np.ascontiguousarray(w3.T, dtype=np.float32)
    in_maps = []
    xh = x.astype(np.float16)
    for core in range(8):
        b, half = core // 2, core % 2
        xq_core = np.ascontiguousarray(x[b][:, half * NQ:(half + 1) * NQ])
        in_maps.append({
            "xkv": xh[b],
            "xq": xq_core,
            "xqh": np.ascontiguousarray(xh[b][:, half * NQ:(half + 1) * NQ]),
            "w1t": w1t.astype(np.float16),
            "w2t": w2t.astype(np.float16),
            "w3t": w3t.astype(np.float16),
        })
    return in_maps


def assemble(results):
    out = np.empty((B, C, N), dtype=np.float32)
    for core in range(8):
        b, half = core // 2, core % 2
        out[b][:, half * NQ:(half + 1) * NQ] = results[core]["out"]
    return out.reshape(B, C, H, W)


def kernel(x, w1, w2, w3):
    nc = _get_nc()
    res = run_bass_kernel_spmd(nc, make_in_maps(x, w1, w2, w3),
                               core_ids=list(range(8)))
    return assemble(res.results)


# revision 19
# speedup vs baseline: 1.1814x; 1.0480x over previous
"""Trainium2 Bass kernel for SAGAN-style spatial self-attention.

Reference computation (per batch b):
    xf = x[b].reshape(C, N)                    # C=256, N=64*64=4096
    f  = w1 @ xf                               # [32, N]   (query^T)
    g  = w2 @ xf                               # [32, N]   (key)
    V  = (w3 @ xf)^T                           # [N, C]    (value)
    S  = f^T @ g                               # [N, N]
    O  = softmax(S, axis=-1) @ V               # [N, C]
    out[b] = O^T.reshape(C, H, W) + x[b]

Sharding: 8 cores = 4 batches x 2 query-halves. Each core holds its batch's
full xf (for keys/values) and computes attention for 2048 query positions.
No cross-core communication.

Per-core device algorithm (n = this core's 2048 query cols, m = all 4096 keys):
  - projections f [32,2048], g [32,4096] in fp16; V [4096,257] in bf16
    (column 256 of V is ones -> PV matmul emits softmax denominator for free)
  - S^T chunks: matmul(lhsT=g_mtile [32,128], rhs=f_chunk [32,512]) -> PSUM
  - P^T = exp(S^T) -> SBUF bf16 (no max subtraction: |S| <~ 45, exp fits fp32)
  - O chunk: matmul(lhsT=P^T [128m,128n], rhs=V [128m,257]) accumulated over
    32 m-tiles -> [128n, 257]; r = 1/col256; O *= r (bf16)
  - DMA-transpose O to [C, n] layout, add residual xq, DMA out.

fp16 (not fp32/fp32r) operands everywhere on the PE: fp32-mode matmuls do not
register as PE-busy for the HAM clock gate and the PE gets stuck at 1.2GHz.
fp16 keeps full clock and has enough mantissa (2^-11) for the pre-exp scores.
"""

import sys

sys.path.insert(0, "/opt/trn_rl_repo")

from contextlib import ExitStack

import numpy as np

import concourse.bass as bass
import concourse.tile as tile
from concourse import bacc, mybir
from concourse.bass import ts, ds
from concourse.bass_utils import run_bass_kernel_spmd

F32 = mybir.dt.float32
F16 = mybir.dt.float16
BF16 = mybir.dt.bfloat16

B, C, H, W = 4, 256, 64, 64
N = H * W          # 4096 keys per batch
NQ = N // 2        # 2048 queries per core
CK = 32            # query/key head dim
MT = N // 128      # 32 m-tiles
NCHUNK = NQ // 512  # 4 n-chunks of 512 query cols
EXP = mybir.ActivationFunctionType.Exp


def build_nc():
    nc = bacc.Bacc("TRN2", target_bir_lowering=False, debug=False, num_devices=8)
    xkv_d = nc.dram_tensor("xkv", [C, N], F16, kind="ExternalInput")
    xq_d = nc.dram_tensor("xq", [C, NQ], F32, kind="ExternalInput")
    xqh_d = nc.dram_tensor("xqh", [C, NQ], F16, kind="ExternalInput")
    w1t_d = nc.dram_tensor("w1t", [C, CK], F16, kind="ExternalInput")
    w2t_d = nc.dram_tensor("w2t", [C, CK], F16, kind="ExternalInput")
    w3t_d = nc.dram_tensor("w3t", [C, C], F16, kind="ExternalInput")
    out_d = nc.dram_tensor("out", [C, NQ], F32, kind="ExternalOutput")

    with tile.TileContext(nc) as tc, ExitStack() as ctx:
        _body(ctx, tc, xkv_d.ap(), xq_d.ap(), xqh_d.ap(), w1t_d.ap(), w2t_d.ap(),
              w3t_d.ap(), out_d.ap())
    nc.compile()
    return nc


def _body(ctx, tc, xkv_d, xq_d, xqh_d, w1t_d, w2t_d, w3t_d, out_d):
    nc = tc.nc
    singles = ctx.enter_context(tc.tile_pool(name="singles", bufs=1))

    xq = singles.tile([128, 2, NQ], F32, tag="xq", name="xq")
    xkv_h = singles.tile([128, 2, N], F16, tag="xkv_h", name="xkv_h")
    xq_h = singles.tile([128, 2, NQ], F16, tag="xq_h", name="xq_h")
    w1t = singles.tile([128, 2, CK], F16, tag="w1t", name="w1t")
    w2t = singles.tile([128, 2, CK], F16, tag="w2t", name="w2t")
    w3t = singles.tile([128, 2, C], F16, tag="w3t", name="w3t")
    g_sb = singles.tile([CK, N], F16, tag="g_sb", name="g_sb")
    f_sb = singles.tile([CK, NQ], F16, tag="f_sb", name="f_sb")
    V = singles.tile([128, MT, 260], BF16, tag="V", name="V")

    nc.vector.memset(V[:, :, 256:257], 1.0)

    # PSUM: one shared pool ("st" tag, 2-bank slots, bufs=3) hosts the S^T
    # tiles and all projection outputs; one 1-bank pool (bufs=2) for the PV
    # accumulators. 6 + 2 = 8 banks.
    stp = ctx.enter_context(tc.tile_pool(name="st_ps", bufs=3, space="PSUM"))
    op = ctx.enter_context(tc.tile_pool(name="o_ps", bufs=2, space="PSUM"))
    ptp = ctx.enter_context(tc.tile_pool(name="pt", bufs=2))
    osbp = ctx.enter_context(tc.tile_pool(name="osb", bufs=2))
    otp = ctx.enter_context(tc.tile_pool(name="ot", bufs=4))
    rp = ctx.enter_context(tc.tile_pool(name="r", bufs=2))
    stgp = ctx.enter_context(tc.tile_pool(name="stage", bufs=3))

    Pt = [None, None]
    stage = [None, None]
    posts = []

    def emit_post(item):
        cc, j, o_ps, stg = item
        J = cc * 4 + j
        r = rp.tile([128, 1], F32, tag="r", name="r")
        nc.vector.reciprocal(r[:], o_ps[:, 256:257])
        o_sb = osbp.tile([128, 256], BF16, tag="osb", name="osb")
        nc.vector.tensor_scalar_mul(o_sb[:], o_ps[:, 0:256], r[:])
        for h in range(2):
            ot = otp.tile([128, 128], BF16, tag="ot", name="ot")
            nc.sync.dma_start_transpose(ot[:], o_sb[:, ts(h, 128)])
            nc.vector.tensor_add(stg[:, h, ts(j, 128)], ot[:],
                                 xq[:, h, ds(J * 128, 128)])
        if j == 3:
            for k in range(2):
                nc.gpsimd.dma_start(out_d[ts(k, 128), ts(cc, 512)], stg[:, k, :])

    def st_group(c, gidx):
        st = stp.tile([128, 2, 512], F32, tag="st", name="st")
        for t in range(2):
            mt = 2 * gidx + t
            nc.tensor.matmul(st[:, t, :], g_sb[:, ts(mt, 128)],
                             f_sb[:, ts(c, 512)], start=True, stop=True)
        nc.scalar.activation(Pt[c % 2][:, 2 * gidx:2 * gidx + 2, :], st[:], EXP)

    # HAM warmup: the PE clock-gate opens only after ~3.4us of gapless
    # streaming; run a dummy dense bf16 burst while the input DMAs land so
    # the projection phase starts at 2.4GHz instead of 1.2GHz.
    warm = singles.tile([128, 512], BF16, tag="warm", name="warm")
    nc.vector.memset(warm[:], 0.0)
    wps = stp.tile([128, 2, 512], F32, tag="st", name="wps")
    for i in range(40):
        nc.tensor.matmul(wps[:, i % 2, :], warm[:, 0:128], warm[:],
                         start=True, stop=True)

    # ---- input DMAs (fp16 operands are cast host-side) ----
    for k in range(2):
        nc.sync.dma_start(w1t[:, k, :], w1t_d[ts(k, 128), :])
    for k in range(2):
        nc.sync.dma_start(xq_h[:, k, 0:512], xqh_d[ts(k, 128), 0:512])
    for k in range(2):
        nc.sync.dma_start(w2t[:, k, :], w2t_d[ts(k, 128), :])
        nc.sync.dma_start(w3t[:, k, :], w3t_d[ts(k, 128), :])
    for ch in range(1, NQ // 512):
        for k in range(2):
            nc.sync.dma_start(xq_h[:, k, ts(ch, 512)], xqh_d[ts(k, 128), ts(ch, 512)])
    for ch in range(N // 512):
        for k in range(2):
            nc.sync.dma_start(xkv_h[:, k, ts(ch, 512)],
                              xkv_d[ts(k, 128), ts(ch, 512)])
    for k in range(2):
        nc.sync.dma_start(xq[:, k, :], xq_d[ts(k, 128), :])

    # ---- projections, interleaved with chunk 0 of the scores (S^T lags the
    # g-projection by one chunk so the PE never waits on the DVE g-copy) ----
    Pt[0] = ptp.tile([128, MT, 512], BF16, tag="pt", name="pt")
    for ch in range(NQ // 512):
        fp = stp.tile([CK, 512], F32, tag="st", name="fp")
        for k in range(2):
            nc.tensor.matmul(fp[:], w1t[:, k, :], xq_h[:, k, ts(ch, 512)],
                             start=(k == 0), stop=(k == 1))
        nc.vector.tensor_copy(f_sb[:, ts(ch, 512)], fp[:])
    # g-projection, V-projection, and chunk-0 scores interleaved in one cycle
    # per 512-col chunk; S^T lags g by one chunk so the PE never waits on the
    # DVE g-copy. The V tiles keep the PE dense while ACT drains the exps.
    for ch in range(N // 512):
        gp = stp.tile([CK, 512], F32, tag="st", name="gp")
        for k in range(2):
            nc.tensor.matmul(gp[:], w2t[:, k, :], xkv_h[:, k, ts(ch, 512)],
                             start=(k == 0), stop=(k == 1))
        nc.vector.tensor_copy(g_sb[:, ts(ch, 512)], gp[:])
        for mt in range(4 * ch, 4 * ch + 4):
            vp = op.tile([128, 256], F32, tag="o", name="vp")
            for k in range(2):
                nc.tensor.matmul(vp[:], xkv_h[:, k, ts(mt, 128)], w3t[:, k, :],
                                 start=(k == 0), stop=(k == 1))
            nc.vector.tensor_copy(V[:, mt, 0:256], vp[:])
        if ch >= 1:
            st_group(0, 2 * (ch - 1))
            st_group(0, 2 * ch - 1)
    st_group(0, 14)
    st_group(0, 15)

    # ---- attention chunks 1..NCHUNK, software-pipelined by one chunk ----
    for c in range(1, NCHUNK + 1):
        if c < NCHUNK:
            Pt[c % 2] = ptp.tile([128, MT, 512], BF16, tag="pt", name="pt")
        stage[(c - 1) % 2] = stgp.tile([128, 2, 512], F32, tag="stage", name="stage")
        o_cur = None
        for gidx in range(16):
            if c < NCHUNK:
                st_group(c, gidx)
            j, seg = gidx // 4, gidx % 4
            if seg == 0:
                o_cur = op.tile([128, 257], F32, tag="o", name="o")
            for mm in range(8):
                mt = seg * 8 + mm
                nc.tensor.matmul(o_cur[:], Pt[(c - 1) % 2][:, mt, ts(j, 128)],
                                 V[:, mt, 0:257],
                                 start=(mt == 0), stop=(mt == MT - 1),
                                 skip_group_check=True)
            if seg == 3:
                posts.append((c - 1, j, o_cur, stage[(c - 1) % 2]))
            # delay each n-tile's post-processing by one PE group so the DVE
            # normalize never stalls the PE stream (flush at the end)
            while len(posts) > (1 if c <= NCHUNK - 1 or gidx < 15 else 0):
                emit_post(posts.pop(0))
    while posts:
        emit_post(posts.pop(0))


_NC_CACHE = None


def _get_nc():
    global _NC_CACHE
    if _NC_CACHE is None:
        _NC_CACHE = build_nc()
    return _NC_CACHE


def make_in_maps(x, w1, w2, w3):
    x = np.ascontiguousarray(x, dtype=np.float32).reshape(B, C, N)
    w1t = np.ascontiguousarray(w1.T, dtype=np.float32)
    w2t = np.ascontiguousarray(w2.T, dtype=np.float32)
    w3t = np.ascontiguousarray(w3.T, dtype=np.float32)
    in_maps = []
    xh = x.astype(np.float16)
    for core in range(8):
        b, half = core // 2, core % 2
        xq_core = np.ascontiguousarray(x[b][:, half * NQ:(half + 1) * NQ])
        in_maps.append({
            "xkv": xh[b],
            "xq": xq_core,
            "xqh": np.ascontiguousarray(xh[b][:, half * NQ:(half + 1) * NQ]),
            "w1t": w1t.astype(np.float16),
            "w2t": w2t.astype(np.float16),
            "w3t": w3t.astype(np.float16),
        })
    return in_maps


def assemble(results):
    out = np.empty((B, C, N), dtype=np.float32)
    for core in range(8):
        b, half = core // 2, core % 2
        out[b][:, half * NQ:(half + 1) * NQ] = results[core]["out"]
    return out.reshape(B, C, H, W)


def kernel(x, w1, w2, w3):
    nc = _get_nc()
    res = run_bass_kernel_spmd(nc, make_in_maps(x, w1, w2, w3),
                               core_ids=list(range(8)))
    return assemble(res.results)


# revision 21
# speedup vs baseline: 1.2022x; 1.0176x over previous
"""Trainium2 Bass kernel for SAGAN-style spatial self-attention.

Reference computation (per batch b):
    xf = x[b].reshape(C, N)                    # C=256, N=64*64=4096
    f  = w1 @ xf                               # [32, N]   (query^T)
    g  = w2 @ xf                               # [32, N]   (key)
    V  = (w3 @ xf)^T                           # [N, C]    (value)
    S  = f^T @ g                               # [N, N]
    O  = softmax(S, axis=-1) @ V               # [N, C]
    out[b] = O^T.reshape(C, H, W) + x[b]

Sharding: 8 cores = 4 batches x 2 query-halves. Each core holds its batch's
full xf (for keys/values) and computes attention for 2048 query positions.
No cross-core communication.

Per-core device algorithm (n = this core's 2048 query cols, m = all 4096 keys):
  - projections f [32,2048], g [32,4096] in fp16; V [4096,257] in bf16
    (column 256 of V is ones -> PV matmul emits softmax denominator for free)
  - S^T chunks: matmul(lhsT=g_mtile [32,128], rhs=f_chunk [32,512]) -> PSUM
  - P^T = exp(S^T) -> SBUF bf16 (no max subtraction: |S| <~ 45, exp fits fp32)
  - O chunk: matmul(lhsT=P^T [128m,128n], rhs=V [128m,257]) accumulated over
    32 m-tiles -> [128n, 257]; r = 1/col256; O *= r (bf16)
  - DMA-transpose O to [C, n] layout, add residual xq, DMA out.

fp16 (not fp32/fp32r) operands everywhere on the PE: fp32-mode matmuls do not
register as PE-busy for the HAM clock gate and the PE gets stuck at 1.2GHz.
fp16 keeps full clock and has enough mantissa (2^-11) for the pre-exp scores.
"""

import sys

sys.path.insert(0, "/opt/trn_rl_repo")

from contextlib import ExitStack

import numpy as np

import concourse.bass as bass
import concourse.tile as tile
from concourse import bacc, mybir
from concourse.bass import ts, ds
from concourse.bass_utils import run_bass_kernel_spmd

F32 = mybir.dt.float32
F16 = mybir.dt.float16
BF16 = mybir.dt.bfloat16

B, C, H, W = 4, 256, 64, 64
N = H * W          # 4096 keys per batch
NQ = N // 2        # 2048 queries per core
CK = 32            # query/key head dim
MT = N // 128      # 32 m-tiles
NCHUNK = NQ // 512  # 4 n-chunks of 512 query cols
EXP = mybir.ActivationFunctionType.Exp


def build_nc():
    nc = bacc.Bacc("TRN2", target_bir_lowering=False, debug=False, num_devices=8)
    xkv_d = nc.dram_tensor("xkv", [C, N], F16, kind="ExternalInput")
    xq_d = nc.dram_tensor("xq", [C, NQ], F32, kind="ExternalInput")
    xqh_d = nc.dram_tensor("xqh", [C, NQ], F16, kind="ExternalInput")
    w1t_d = nc.dram_tensor("w1t", [C, CK], F16, kind="ExternalInput")
    w2t_d = nc.dram_tensor("w2t", [C, CK], F16, kind="ExternalInput")
    w3t_d = nc.dram_tensor("w3t", [C, C], F16, kind="ExternalInput")
    out_d = nc.dram_tensor("out", [C, NQ], F32, kind="ExternalOutput")

    with tile.TileContext(nc) as tc, ExitStack() as ctx:
        _body(ctx, tc, xkv_d.ap(), xq_d.ap(), xqh_d.ap(), w1t_d.ap(), w2t_d.ap(),
              w3t_d.ap(), out_d.ap())
    nc.compile()
    return nc


def _body(ctx, tc, xkv_d, xq_d, xqh_d, w1t_d, w2t_d, w3t_d, out_d):
    nc = tc.nc
    singles = ctx.enter_context(tc.tile_pool(name="singles", bufs=1))

    xq = singles.tile([128, 2, NQ], F32, tag="xq", name="xq")
    xkv_h = singles.tile([128, 2, N], F16, tag="xkv_h", name="xkv_h")
    xq_h = singles.tile([128, 2, NQ], F16, tag="xq_h", name="xq_h")
    w1t = singles.tile([128, 2, CK], F16, tag="w1t", name="w1t")
    w2t = singles.tile([128, 2, CK], F16, tag="w2t", name="w2t")
    w3t = singles.tile([128, 2, C], F16, tag="w3t", name="w3t")
    g_sb = singles.tile([CK, N], F16, tag="g_sb", name="g_sb")
    f_sb = singles.tile([CK, NQ], F16, tag="f_sb", name="f_sb")
    V = singles.tile([128, MT, 260], BF16, tag="V", name="V")

    nc.vector.memset(V[:, :, 256:257], 1.0)

    # PSUM: one shared pool ("st" tag, 2-bank slots, bufs=3) hosts the S^T
    # tiles and all projection outputs; one 1-bank pool (bufs=2) for the PV
    # accumulators. 6 + 2 = 8 banks.
    stp = ctx.enter_context(tc.tile_pool(name="st_ps", bufs=3, space="PSUM"))
    op = ctx.enter_context(tc.tile_pool(name="o_ps", bufs=2, space="PSUM"))
    ptp = ctx.enter_context(tc.tile_pool(name="pt", bufs=2))
    osbp = ctx.enter_context(tc.tile_pool(name="osb", bufs=2))
    otp = ctx.enter_context(tc.tile_pool(name="ot", bufs=4))
    rp = ctx.enter_context(tc.tile_pool(name="r", bufs=2))
    stgp = ctx.enter_context(tc.tile_pool(name="stage", bufs=3))

    Pt = [None, None]
    stage = [None, None]
    posts = []

    def emit_post(item):
        cc, j, o_ps, stg = item
        J = cc * 4 + j
        r = rp.tile([128, 1], F32, tag="r", name="r")
        nc.vector.reciprocal(r[:], o_ps[:, 256:257])
        o_sb = osbp.tile([128, 256], BF16, tag="osb", name="osb")
        nc.vector.tensor_scalar_mul(o_sb[:], o_ps[:, 0:256], r[:])
        for h in range(2):
            ot = otp.tile([128, 128], BF16, tag="ot", name="ot")
            eng = nc.sync if h == 0 else nc.scalar
            eng.dma_start_transpose(ot[:], o_sb[:, ts(h, 128)])
            nc.vector.tensor_add(stg[:, h, ts(j, 128)], ot[:],
                                 xq[:, h, ds(J * 128, 128)])
        if j == 3:
            for k in range(2):
                nc.gpsimd.dma_start(out_d[ts(k, 128), ts(cc, 512)], stg[:, k, :])

    def st_group(c, gidx):
        st = stp.tile([128, 2, 512], F32, tag="st", name="st")
        for t in range(2):
            mt = 2 * gidx + t
            nc.tensor.matmul(st[:, t, :], g_sb[:, ts(mt, 128)],
                             f_sb[:, ts(c, 512)], start=True, stop=True)
        nc.scalar.activation(Pt[c % 2][:, 2 * gidx:2 * gidx + 2, :], st[:], EXP)

    # HAM warmup: the PE clock-gate opens only after ~3.4us of gapless
    # streaming; run a dummy dense bf16 burst while the input DMAs land so
    # the projection phase starts at 2.4GHz instead of 1.2GHz.
    warm = singles.tile([128, 512], BF16, tag="warm", name="warm")
    nc.vector.memset(warm[:], 0.0)
    wps = stp.tile([128, 2, 512], F32, tag="st", name="wps")
    for i in range(40):
        nc.tensor.matmul(wps[:, i % 2, :], warm[:, 0:128], warm[:],
                         start=True, stop=True)

    # ---- input DMAs (fp16 operands are cast host-side) ----
    for k in range(2):
        nc.sync.dma_start(w1t[:, k, :], w1t_d[ts(k, 128), :])
    for k in range(2):
        nc.sync.dma_start(xq_h[:, k, 0:512], xqh_d[ts(k, 128), 0:512])
    for k in range(2):
        nc.sync.dma_start(w2t[:, k, :], w2t_d[ts(k, 128), :])
        nc.sync.dma_start(w3t[:, k, :], w3t_d[ts(k, 128), :])
    for ch in range(1, NQ // 512):
        for k in range(2):
            nc.sync.dma_start(xq_h[:, k, ts(ch, 512)], xqh_d[ts(k, 128), ts(ch, 512)])
    for ch in range(N // 512):
        for k in range(2):
            nc.sync.dma_start(xkv_h[:, k, ts(ch, 512)],
                              xkv_d[ts(k, 128), ts(ch, 512)])
    for k in range(2):
        nc.sync.dma_start(xq[:, k, :], xq_d[ts(k, 128), :])

    # ---- projections, interleaved with chunk 0 of the scores (S^T lags the
    # g-projection by one chunk so the PE never waits on the DVE g-copy) ----
    Pt[0] = ptp.tile([128, MT, 512], BF16, tag="pt", name="pt")
    for ch in range(NQ // 512):
        fp = stp.tile([CK, 512], F32, tag="st", name="fp")
        for k in range(2):
            nc.tensor.matmul(fp[:], w1t[:, k, :], xq_h[:, k, ts(ch, 512)],
                             start=(k == 0), stop=(k == 1))
        nc.vector.tensor_copy(f_sb[:, ts(ch, 512)], fp[:])
    # g-projection, V-projection, and chunk-0 scores interleaved in one cycle
    # per 512-col chunk; S^T lags g by one chunk so the PE never waits on the
    # DVE g-copy. The V tiles keep the PE dense while ACT drains the exps.
    for ch in range(N // 512):
        if ch >= 1:
            st_group(0, 2 * (ch - 1))
            st_group(0, 2 * ch - 1)
        gp = stp.tile([CK, 512], F32, tag="st", name="gp")
        for k in range(2):
            nc.tensor.matmul(gp[:], w2t[:, k, :], xkv_h[:, k, ts(ch, 512)],
                             start=(k == 0), stop=(k == 1))
        nc.vector.tensor_copy(g_sb[:, ts(ch, 512)], gp[:])
        for mt in range(4 * ch, 4 * ch + 4):
            vp = op.tile([128, 256], F32, tag="o", name="vp")
            for k in range(2):
                nc.tensor.matmul(vp[:], xkv_h[:, k, ts(mt, 128)], w3t[:, k, :],
                                 start=(k == 0), stop=(k == 1))
            nc.vector.tensor_copy(V[:, mt, 0:256], vp[:])
    st_group(0, 14)
    st_group(0, 15)

    # ---- attention chunks 1..NCHUNK, software-pipelined by one chunk ----
    for c in range(1, NCHUNK + 1):
        if c < NCHUNK:
            Pt[c % 2] = ptp.tile([128, MT, 512], BF16, tag="pt", name="pt")
        stage[(c - 1) % 2] = stgp.tile([128, 2, 512], F32, tag="stage", name="stage")
        o_cur = None
        for gidx in range(16):
            if c < NCHUNK:
                st_group(c, gidx)
            j, seg = gidx // 4, gidx % 4
            if seg == 0:
                o_cur = op.tile([128, 257], F32, tag="o", name="o")
            for mm in range(8):
                mt = seg * 8 + mm
                nc.tensor.matmul(o_cur[:], Pt[(c - 1) % 2][:, mt, ts(j, 128)],
                                 V[:, mt, 0:257],
                                 start=(mt == 0), stop=(mt == MT - 1),
                                 skip_group_check=True)
            if seg == 3:
                posts.append((c - 1, j, o_cur, stage[(c - 1) % 2]))
            # delay each n-tile's post-processing by one PE group so the DVE
            # normalize never stalls the PE stream (flush at the end)
            while len(posts) > (1 if c <= NCHUNK - 1 or gidx < 15 else 0):
                emit_post(posts.pop(0))
    while posts:
        emit_post(posts.pop(0))


_NC_CACHE = None


def _get_nc():
    global _NC_CACHE
    if _NC_CACHE is None:
        _NC_CACHE = build_nc()
    return _NC_CACHE


def make_in_maps(x, w1, w2, w3):
    x = np.ascontiguousarray(x, dtype=np.float32).reshape(B, C, N)
    w1t = np.ascontiguousarray(w1.T, dtype=np.float32)
    w2t = np.ascontiguousarray(w2.T, dtype=np.float32)
    w3t = np.ascontiguousarray(w3.T, dtype=np.float32)
    in_maps = []
    xh = x.astype(np.float16)
    for core in range(8):
        b, half = core // 2, core % 2
        xq_core = np.ascontiguousarray(x[b][:, half * NQ:(half + 1) * NQ])
        in_maps.append({
            "xkv": xh[b],
            "xq": xq_core,
            "xqh": np.ascontiguousarray(xh[b][:, half * NQ:(half + 1) * NQ]),
            "w1t": w1t.astype(np.float16),
            "w2t": w2t.astype(np.float16),
            "w3t": w3t.astype(np.float16),
        })
    return in_maps


def assemble(results):
    out = np.empty((B, C, N), dtype=np.float32)
    for core in range(8):
        b, half = core // 2, core % 2
        out[b][:, half * NQ:(half + 1) * NQ] = results[core]["out"]
    return out.reshape(B, C, H, W)


def kernel(x, w1, w2, w3):
    nc = _get_nc()
    res = run_bass_kernel_spmd(nc, make_in_maps(x, w1, w2, w3),
                               core_ids=list(range(8)))
    return assemble(res.results)
